# revision 1
# baseline (speedup 1.0000x reference)
"""NetVLAD Trainium2 kernel (8 NeuronCores, batch-per-core sharding).

Strategy:
  - Host: stable-sort points by batch_id; core i owns batch i entirely,
    padded to T*128 rows (shared T; pads are e0 unit vectors, corrected
    exactly post-aggregation). Feat shipped twice in bf16 (natural +
    transposed) = same bytes as one fp32 copy.
  - Device main loop (groups of G=16 tiles of 128 points):
      ssq per tile: fused square-accum split across DVE (STT) / ACT (Square)
      r = exp(-0.5*ln(ssq)) on ACT
      logits: 2 bf16 matmuls per tile into shared PSUM banks (8 tiles/bank)
      rowmax straight from PSUM (batched DVE reduce, negated)
      e' = exp(r*psum + (-r*max + 35)) from PSUM (ACT, scale/bias APs), bf16
      e'' = e' * E_rep on GpSimd (E[k] = exp(conv_b[k] - max conv_b + 35);
            exact softmax refactoring — the conv bias never enters the
            matmul or a per-tile STT; the +-35 shifts keep e' in fp32 and
            E/e'' in normal bf16 for conv_b spread <= ~125)
      Z = rowsum(e''), s2 = r/Z
      soft2 = bf16(e'' * s2) in one batched GpSimd op (stride-0 broadcast);
      agg[:,0:256] += soft2^T @ featN and agg[:,256] += soft2^T @ ||feat||
      (N=1 matmul, stationary weights reused) into one of two [64,257]
      PSUM banks (first/second half of the loop)
  - Half-1 agg is AllGathered (fp16) mid-loop, fully hidden; only the
    half-2 AllGather is exposed at loop end.  Every core then combines,
    pad-corrects, builds negated VLAD (S*c - A), intra-normalizes all 8
    batches, transposes via PE into FC operand layout, computes its
    128-col FC output slice (negated fc_w), AllGathers the [8,129]
    slices+partial norms, and applies the final l2norm.
"""

import numpy as np
import ml_dtypes

BF16 = ml_dtypes.bfloat16

N, C, K, B, OUT = 200000, 256, 64, 8, 1024
NCORES = 8
P = 128
G = 16                # tiles per group
GB = 8                # tiles per PSUM logits bank
# exp shift, split between the e' bias (+35) and the host constant E (+35):
# keeps e' in normal fp32 and E/e'' in normal bf16 given conv_b spread <= ~125
SHIFT = 35.0

_compiled_cache = {}
PROFILE = False
LAST_RESULT = None


# ----------------------------------------------------------------------------
# Host-side planning
# ----------------------------------------------------------------------------

def _plan(feat, batch_ids):
    """Sort by batch; core i gets batch i padded to T*128 rows (shared T)."""
    order = np.argsort(batch_ids, kind="stable")
    feat_s = feat[order]
    counts = np.bincount(batch_ids, minlength=B)
    T = int(np.ceil(counts.max() / P))
    n_pad = [T * P - int(c) for c in counts]

    pad_row = np.zeros((C,), np.float32)
    pad_row[0] = 1.0

    core_feat = []
    off = 0
    for b in range(B):
        nb = int(counts[b])
        fb = feat_s[off:off + nb]
        off += nb
        if n_pad[b]:
            fb = np.concatenate([fb, np.broadcast_to(pad_row, (n_pad[b], C))], 0)
        core_feat.append(fb)
    return core_feat, T, n_pad


def _pad_correction(conv_w, conv_b, n_pad):
    """Exact contribution of one e0 pad row through the device pipeline."""
    w_bf = conv_w.astype(BF16)
    raw = w_bf[:, 0].astype(np.float32)          # feat=e0 -> raw logits
    # r = exp(-0.5*ln(1.0)) = 1 exactly (ssq = 1)
    r = np.float32(1.0)
    m = raw.max()
    E = np.exp(conv_b.astype(np.float32) - np.float32(conv_b.max())
               + np.float32(SHIFT)).astype(BF16)
    e1 = np.exp(r * raw - r * m + np.float32(SHIFT)).astype(np.float32)
    e2 = (e1 * E.astype(np.float32)).astype(BF16)
    Z = e2.astype(np.float32).sum()
    s2 = r * (np.float32(1.0) / Z)
    # device computes bf16(e2*s2)^T @ [featN, nrm]; pad cols are exactly 1.0
    soft2 = (e2.astype(np.float32) * s2).astype(BF16).astype(np.float32)
    # one pad row contributes soft2[k] at col0 (feat e0) and col C (norm=1)
    corr = np.zeros((K, 2), np.float32)
    corr[:, 0] = n_pad * soft2
    corr[:, 1] = n_pad * soft2
    return corr


# ----------------------------------------------------------------------------
# Device program
# ----------------------------------------------------------------------------

def _build_nc(T):
    import concourse.bass as bass
    import concourse.bacc as bacc
    import concourse.mybir as mybir
    from concourse import tile

    dt = mybir.dt
    AF = mybir.ActivationFunctionType
    ALU = mybir.AluOpType


    NP = T * P
    OSL = OUT // NCORES  # 128 output cols per core


    nc = bacc.Bacc(
        "TRN2", target_bir_lowering=False, debug=False, num_devices=NCORES
    )

    # --- I/O ---
    featN_d = nc.dram_tensor("featN", [P, T, C], dt.bfloat16, kind="ExternalInput").ap()
    featT_d = nc.dram_tensor("featT", [C, NP], dt.bfloat16, kind="ExternalInput").ap()
    wt_d = nc.dram_tensor("wt", [C, K], dt.bfloat16, kind="ExternalInput").ap()
    erep_d = nc.dram_tensor("erep", [P, G * K], dt.bfloat16, kind="ExternalInput").ap()
    cent_d = nc.dram_tensor("cent", [P, C], dt.bfloat16, kind="ExternalInput").ap()
    corr_d = nc.dram_tensor("corr", [B * K, 2], dt.float32, kind="ExternalInput").ap()
    fwt_d = nc.dram_tensor("fwt", [P, K * C], dt.bfloat16, kind="ExternalInput").ap()
    fbb_d = nc.dram_tensor("fbb", [B, OSL], dt.float32, kind="ExternalInput").ap()
    ident_d = nc.dram_tensor("ident", [P, P], dt.bfloat16, kind="ExternalInput").ap()
    sel_d = nc.dram_tensor("sel", [P, B], dt.float32, kind="ExternalInput").ap()
    out_d = nc.dram_tensor("out", [B, OUT], dt.float32, kind="ExternalOutput").ap()

    featT_v = featT_d.rearrange("(h p) n -> p h n", h=2)

    with tile.TileContext(nc) as tc:
        with (
            tc.tile_pool(name="const", bufs=1) as cpool,
            tc.tile_pool(name="dram", bufs=1, space="DRAM") as dram,
        ):
            wt_sb = cpool.tile([P, 2, K], dt.bfloat16, name="wt_sb")
            for h in range(2):
                nc.sync.dma_start(out=wt_sb[:, h, :], in_=wt_d[h * P:(h + 1) * P, :])
            erep_sb = cpool.tile([P, G * K], dt.bfloat16, name="erep_sb")
            nc.sync.dma_start(out=erep_sb[:, :], in_=erep_d[:, :])
            # tail-only constants: tiles declared here, DMAs deferred into the
            # loop so the first feat groups win the DMA queues
            cent_sb = cpool.tile([P, C], dt.bfloat16, name="cent_sb")
            corr_sb = cpool.tile([P, 4, 2], dt.float32, name="corr_sb")
            avA = cpool.tile([P, 4, C + 1], dt.float16, name="avA")
            ident_sb = cpool.tile([P, P], dt.bfloat16, name="ident_sb")
            fbb_sb = cpool.tile([B, OSL], dt.float32, name="fbb_sb")
            sel_sb = cpool.tile([P, B], dt.float32, name="sel_sb")
            fwt_sb = cpool.tile([P, K * C], dt.bfloat16, name="fwt_sb")

            def _load_tail_consts():
                nc.sync.dma_start(out=cent_sb[:, :], in_=cent_d[:, :])
                nc.sync.dma_start(
                    out=corr_sb[:, :, :],
                    in_=corr_d.rearrange("(q p) c -> p q c", q=4))
                nc.sync.dma_start(out=ident_sb[:, :], in_=ident_d[:, :])
                nc.sync.dma_start(out=fbb_sb[:, :], in_=fbb_d[:, :])
                nc.sync.dma_start(out=sel_sb[:, :], in_=sel_d[:, :])

            def _load_fwt_chunk(q):
                qs = K * C // 8
                eng = nc.sync if q % 2 == 0 else nc.scalar
                eng.dma_start(out=fwt_sb[:, q * qs:(q + 1) * qs],
                              in_=fwt_d[:, q * qs:(q + 1) * qs])

            # ---------------- main point loop ----------------
            with (
                tc.tile_pool(name="aggp", bufs=1, space="PSUM") as aggp,
                tc.tile_pool(name="psl", bufs=2, space="PSUM") as pslp,
                tc.tile_pool(name="feed", bufs=4) as fepool,
                tc.tile_pool(name="grp", bufs=2) as gpool,
                tc.tile_pool(name="tl", bufs=3) as tpool,
            ):
                aggA = aggp.tile([K, C + 1], dt.float32, name="aggA")
                aggB = aggp.tile([K, C + 1], dt.float32, name="aggB")
                NGRP = (T + G - 1) // G
                HALF = (NGRP // 2) * G  # first tile of bank B

                ag_a_in = dram.tile([K, C + 1], dt.float16, name="ag_a_in")
                ag_a_out = dram.tile([NCORES * K, C + 1], dt.float16,
                                     name="ag_a_out")
                ag_b_in = dram.tile([K, C + 1], dt.float16, name="ag_b_in")
                ag_b_out = dram.tile([NCORES * K, C + 1], dt.float16,
                                     name="ag_b_out")
                evA = cpool.tile([K, C + 1], dt.float16, name="evA")

                t = 0
                gi = 0
                while t < T:
                    if gi == 2:
                        _load_tail_consts()
                    if 2 <= gi < 10:
                        _load_fwt_chunk(gi - 2)
                    if t == HALF:
                        # bank A complete: evac + AllGather (hidden by loop)
                        nc.scalar.copy(evA[:, :], aggA[:, :])
                        nc.sync.dma_start(out=ag_a_in[:, :], in_=evA[:, :])
                        nc.gpsimd.collective_compute(
                            "AllGather",
                            ALU.bypass,
                            replica_groups=[list(range(NCORES))],
                            ins=[ag_a_in[:, :]],
                            outs=[ag_a_out[:, :]],
                        )
                    gi += 1
                    g_size = min(G, T - t)
                    nbank = (g_size + GB - 1) // GB
                    featT_g = fepool.tile([P, 2, G * P], dt.bfloat16, name="featT_g")
                    featN_g = fepool.tile([P, G, C], dt.bfloat16, name="featN_g")
                    e1_g = gpool.tile([P, G * K], dt.bfloat16, name="e1_g")
                    e2_g = gpool.tile([P, G * K], dt.bfloat16, name="e2_g")
                    soft2_g = gpool.tile([P, G, K], dt.bfloat16, name="soft2_g")
                    ssq_g = gpool.tile([P, G], dt.float32, name="ssq_g")
                    nrm_g = gpool.tile([P, G], dt.bfloat16, name="nrm_g")
                    lns_g = gpool.tile([P, G], dt.float32, name="lns_g")
                    r_g = gpool.tile([P, G], dt.float32, name="r_g")
                    negm_g = gpool.tile([P, G], dt.float32, name="negm_g")
                    bias_g = gpool.tile([P, G], dt.float32, name="bias_g")
                    z_g = gpool.tile([P, G], dt.float32, name="z_g")
                    rz_g = gpool.tile([P, G], dt.float32, name="rz_g")
                    s2_g = gpool.tile([P, G], dt.float32, name="s2_g")

                    # one DMA each: featT slab + featN slab
                    nc.sync.dma_start(
                        out=featT_g[:, :, 0:g_size * P],
                        in_=featT_v[:, :, t * P:(t + g_size) * P],
                    )
                    nc.scalar.dma_start(
                        out=featN_g[:, 0:g_size, :],
                        in_=featN_d[:, t:t + g_size, :],
                    )

                    # ssq per tile: DVE fused square-accum (11) / ACT square (5)
                    for g in range(g_size):
                        scr = tpool.tile([P, C], dt.bfloat16, name="scr",
                                         tag="scr", bufs=4)
                        if g % 3 == 2 or g % 8 == 0:
                            nc.scalar.activation(
                                scr[:, :], featN_g[:, g, :], AF.Square,
                                accum_out=ssq_g[:, g:g + 1],
                            )
                        else:
                            nc.vector.scalar_tensor_tensor(
                                out=scr[:, :],
                                in0=featN_g[:, g, :],
                                scalar=1.0,
                                in1=featN_g[:, g, :],
                                op0=ALU.mult,
                                op1=ALU.mult,
                                accum_out=ssq_g[:, g:g + 1],
                            )
                    # r = exp(-0.5*ln(ssq))
                    nc.scalar.activation(
                        lns_g[:, 0:g_size], ssq_g[:, 0:g_size], AF.Ln)
                    nc.scalar.activation(
                        r_g[:, 0:g_size], lns_g[:, 0:g_size], AF.Exp,
                        scale=-0.5)
                    # norm = ssq*r (scaled by s2 later into the fsc norm col)
                    nc.vector.tensor_tensor(
                        out=nrm_g[:, 0:g_size],
                        in0=ssq_g[:, 0:g_size],
                        in1=r_g[:, 0:g_size],
                        op=ALU.mult,
                    )

                    # logits matmuls into shared PSUM banks (GB tiles each)
                    banks = [pslp.tile([P, GB * K], dt.float32, name=f"bank{i}",
                                       tag=f"bank{i}") for i in range(nbank)]
                    for g in range(g_size):
                        bk, sl = banks[g // GB], (g % GB) * K
                        nc.tensor.matmul(
                            bk[:, sl:sl + K],
                            lhsT=featT_g[:, 0, g * P:(g + 1) * P],
                            rhs=wt_sb[:, 0, :],
                            start=True, stop=False,
                        )
                        nc.tensor.matmul(
                            bk[:, sl:sl + K],
                            lhsT=featT_g[:, 1, g * P:(g + 1) * P],
                            rhs=wt_sb[:, 1, :],
                            start=False, stop=True,
                        )

                    # batched negated row max from PSUM, bias = -r*max + SHIFT
                    for i in range(nbank):
                        lo = i * GB
                        n_in = min(GB, g_size - lo)
                        nc.vector.tensor_reduce(
                            out=negm_g[:, lo:lo + n_in],
                            in_=banks[i].rearrange("p (g k) -> p g k", k=K)[:, 0:n_in, :],
                            axis=mybir.AxisListType.X,
                            op=ALU.max,
                            negate=True,
                        )
                    # bias = negm*r + SHIFT  (TT then TS)
                    nc.vector.tensor_tensor(
                        out=bias_g[:, 0:g_size],
                        in0=negm_g[:, 0:g_size],
                        in1=r_g[:, 0:g_size],
                        op=ALU.mult,
                    )
                    nc.vector.tensor_scalar(
                        out=bias_g[:, 0:g_size],
                        in0=bias_g[:, 0:g_size],
                        scalar1=SHIFT,
                        scalar2=None,
                        op0=ALU.add,
                    )

                    # e' = exp(r*psum + bias) per tile, straight from PSUM
                    for g in range(g_size):
                        bk, sl = banks[g // GB], (g % GB) * K
                        nc.scalar.activation(
                            e1_g[:, g * K:(g + 1) * K],
                            bk[:, sl:sl + K],
                            AF.Exp,
                            bias=bias_g[:, g:g + 1],
                            scale=r_g[:, g:g + 1],
                        )
                    # e'' = e' * E_rep (gpsimd TT), Z = rowsum (DVE)
                    nc.gpsimd.tensor_tensor(
                        out=e2_g[:, 0:g_size * K],
                        in0=e1_g[:, 0:g_size * K],
                        in1=erep_sb[:, 0:g_size * K],
                        op=ALU.mult,
                    )
                    nc.vector.tensor_reduce(
                        out=z_g[:, 0:g_size],
                        in_=e2_g.rearrange("p (g k) -> p g k", k=K)[:, 0:g_size, :],
                        axis=mybir.AxisListType.X,
                        op=ALU.add,
                    )
                    nc.vector.reciprocal(rz_g[:, 0:g_size], z_g[:, 0:g_size])
                    nc.vector.tensor_tensor(
                        out=s2_g[:, 0:g_size],
                        in0=r_g[:, 0:g_size],
                        in1=rz_g[:, 0:g_size],
                        op=ALU.mult,
                    )
                    # soft2 = e2 * s2 (one batched GpSimd op, s2 broadcast)
                    nc.gpsimd.tensor_tensor(
                        out=soft2_g[:, 0:g_size, :],
                        in0=e2_g.rearrange("p (g k) -> p g k", k=K)[:, 0:g_size, :],
                        in1=s2_g[:, 0:g_size].rearrange("p g -> p g ()")
                            .broadcast_to([P, g_size, K]),
                        op=ALU.mult,
                    )
                    # agg: feat part (N=256) + norm col (N=1, weights reused)
                    for g in range(g_size):
                        tt = t + g
                        agg = aggA if tt < HALF else aggB
                        st = (tt == 0 or tt == HALF)
                        sp = (tt == HALF - 1 or tt == T - 1)
                        nc.tensor.matmul(
                            agg[:, 0:C],
                            lhsT=soft2_g[:, g, :],
                            rhs=featN_g[:, g, :],
                            start=st, stop=sp,
                        )
                        nc.tensor.matmul(
                            agg[:, C:C + 1],
                            lhsT=soft2_g[:, g, :],
                            rhs=nrm_g[:, g:g + 1],
                            start=st, stop=sp,
                        )
                    t += g_size

            # ---------------- tail: vlad, AG, fc, AG, norm ----------------
            with (
                tc.tile_pool(name="fin", bufs=1) as fpool,
                tc.tile_pool(name="fps", bufs=2, space="PSUM") as fpsum,
                tc.tile_pool(name="fcp", bufs=1, space="PSUM") as fcps,
            ):
                # evac bank B, AllGather it
                evB = fpool.tile([K, C + 1], dt.float16, name="evB")
                nc.scalar.copy(evB[:, :], aggB[:, :])
                nc.sync.dma_start(out=ag_b_in[:, :], in_=evB[:, :])
                nc.gpsimd.collective_compute(
                    "AllGather",
                    ALU.bypass,
                    replica_groups=[list(range(NCORES))],
                    ins=[ag_b_in[:, :]],
                    outs=[ag_b_out[:, :]],
                )
                # combine halves, pad-correct, vlad, intra-normalize:
                # 4 tiles of 128 rows, tile q = batches {2q, 2q+1}
                # avA stages during the AG-b wait (depends only on AG-a)
                nc.sync.dma_start(
                    out=avA[:, :, :],
                    in_=ag_a_out.rearrange("(q p) c -> p q c", q=4))
                avB = fpool.tile([P, 4, C + 1], dt.float16, name="avB")
                nc.sync.dma_start(
                    out=avB[:, :, :],
                    in_=ag_b_out.rearrange("(q p) c -> p q c", q=4))
                ssv = fpool.tile([P, 4], dt.float32, name="ssv")
                lnv = fpool.tile([P, 4], dt.float32, name="lnv")
                rnv = fpool.tile([P, 4], dt.float32, name="rnv")
                vT_all = fpool.tile([P, 2, B, K], dt.bfloat16, name="vT_all")
                avS = fpool.tile([P, 4, C + 1], dt.float32, name="avS")
                nc.vector.tensor_tensor(
                    out=avS[:, :, :], in0=avA[:, :, :], in1=avB[:, :, :],
                    op=ALU.add)
                # pad correction touches only cols 0 and C
                nc.vector.tensor_tensor(
                    out=avS[:, :, 0], in0=avS[:, :, 0], in1=corr_sb[:, :, 0],
                    op=ALU.subtract)
                nc.vector.tensor_tensor(
                    out=avS[:, :, C], in0=avS[:, :, C], in1=corr_sb[:, :, 1],
                    op=ALU.subtract)
                nvq = []
                for q in range(4):
                    nv = fpool.tile([P, C], dt.float32, name="nv", tag="nv",
                                    bufs=4)
                    nvq.append(nv)
                    nc.vector.scalar_tensor_tensor(
                        out=nv[:, :], in0=cent_sb[:, :],
                        scalar=avS[:, q, C:C + 1], in1=avS[:, q, 0:C],
                        op0=ALU.mult, op1=ALU.subtract)
                    nvs = fpool.tile([P, C], dt.float32, name="nvs", tag="nvs",
                                     bufs=2)
                    if q % 2 == 1:
                        nc.scalar.activation(
                            nvs[:, :], nv[:, :], AF.Square,
                            accum_out=ssv[:, q:q + 1])
                    else:
                        nc.vector.scalar_tensor_tensor(
                            out=nvs[:, :], in0=nv[:, :], scalar=1.0, in1=nv[:, :],
                            op0=ALU.mult, op1=ALU.mult, accum_out=ssv[:, q:q + 1])
                nc.vector.tensor_scalar_max(ssv[:, :], ssv[:, :], 1e-24)
                nc.scalar.activation(lnv[:, :], ssv[:, :], AF.Ln)
                nc.scalar.activation(rnv[:, :], lnv[:, :], AF.Exp, scale=-0.5)
                ptb = [fpsum.tile([P, 4 * P], dt.bfloat16, name=f"ptb{h}",
                                  bufs=1) for h in range(2)]
                for q in range(4):
                    vbf = fpool.tile([P, C], dt.bfloat16, name="vbf",
                                     tag="vbf", bufs=2)
                    nc.vector.tensor_scalar(
                        out=vbf[:, :], in0=nvq[q][:, :],
                        scalar1=rnv[:, q:q + 1], scalar2=None, op0=ALU.mult)
                    for h in range(2):
                        nc.tensor.transpose(
                            ptb[h][:, q * P:(q + 1) * P],
                            vbf[:, h * P:(h + 1) * P],
                            ident_sb[:, :])
                # contiguous evacs; bank col layout is already 64*b + k, so
                # the FC reads lhsT with a single stride-64 AP — no permute
                for h in range(2):
                    nc.vector.tensor_copy(vT_all[:, h, :, :], ptb[h][:, :])

                # FC: out[8b, 128o] in 4 concurrent col-groups, separate banks
                # chunk j=(h,k): lhsT = vT_all[:, :, h, (j k)] -> [128, 4, 2]
                fcpg = [fcps.tile([P, OSL], dt.float32, name=f"fcp{gq}", bufs=1)
                        for gq in range(4)]
                NCH = K * C // P  # 128
                for j in range(NCH):
                    grp = j % 4
                    h, k = j % 2, j // 2
                    nc.tensor.matmul(
                        fcpg[grp][32 * grp:32 * grp + B, :],
                        lhsT=vT_all[:, h, :, k],
                        rhs=fwt_sb[:, j * OSL:(j + 1) * OSL],
                        start=(j < 4), stop=(j >= NCH - 4),
                        tile_position=(0, 32 * grp),
                        skip_group_check=True,
                    )
                sb4 = fpool.tile([P, OSL], dt.float32, name="sb4")
                nc.vector.memset(sb4[:, :], 0.0)
                for gq in range(4):
                    nc.scalar.copy(
                        sb4[32 * gq:32 * gq + B, :],
                        fcpg[gq][32 * gq:32 * gq + B, :])
                fcsum = fcps.tile([P, OSL], dt.float32, name="fcsum", bufs=1)
                nc.tensor.matmul(
                    fcsum[0:B, :], lhsT=sel_sb[:, :], rhs=sb4[:, :],
                    start=True, stop=True, skip_group_check=True,
                )
                fo = fpool.tile([B, OSL], dt.float32, name="fo")
                nc.vector.tensor_tensor(
                    out=fo[:, :], in0=fcsum[0:B, :], in1=fbb_sb[:, :],
                    op=ALU.add)

                # AllGather the [8, 128] slices + per-core partial sumsq
                fop = fpool.tile([B, OSL + 1], dt.float32, name="fop")
                nc.vector.scalar_tensor_tensor(
                    out=fop[:, 0:OSL], in0=fo[:, :], scalar=1.0,
                    in1=fo[:, :], op0=ALU.mult, op1=ALU.mult,
                    accum_out=fop[:, OSL:OSL + 1])
                nc.vector.tensor_copy(fop[:, 0:OSL], fo[:, :])
                ag_in = dram.tile([B, OSL + 1], dt.float32, name="ag_in")
                ag_out = dram.tile([NCORES * B, OSL + 1], dt.float32, name="ag_out")
                nc.sync.dma_start(out=ag_in[:, :], in_=fop[:, :])
                nc.gpsimd.collective_compute(
                    "AllGather",
                    ALU.bypass,
                    replica_groups=[list(range(NCORES))],
                    ins=[ag_in[:, :]],
                    outs=[ag_out[:, :]],
                )
                fin = fpool.tile([B, OUT], dt.float32, name="fin")
                agv = ag_out.rearrange("(c b) o -> b c o", b=B)
                nc.sync.dma_start(
                    out=fin.rearrange("b (c o) -> b c o", c=NCORES),
                    in_=agv[:, :, 0:OSL],
                )
                ssfp = fpool.tile([B, NCORES], dt.float32, name="ssfp")
                nc.sync.dma_start(out=ssfp[:, :], in_=agv[:, :, OSL])
                ssf = fpool.tile([B, 1], dt.float32, name="ssf")
                lnf = fpool.tile([B, 1], dt.float32, name="lnf")
                rnf = fpool.tile([B, 1], dt.float32, name="rnf")
                nc.vector.tensor_reduce(
                    out=ssf[:, :], in_=ssfp[:, :],
                    axis=mybir.AxisListType.X, op=ALU.add)
                nc.vector.tensor_scalar_max(ssf[:, :], ssf[:, :], 1e-24)
                nc.scalar.activation(lnf[:, :], ssf[:, :], AF.Ln)
                nc.scalar.activation(rnf[:, :], lnf[:, :], AF.Exp, scale=-0.5)
                fout = fpool.tile([B, OUT], dt.float32, name="fout")
                nc.vector.tensor_scalar(
                    out=fout[:, :], in0=fin[:, :],
                    scalar1=rnf[:, 0:1], scalar2=None, op0=ALU.mult)
                nc.sync.dma_start(out=out_d[:, :], in_=fout[:, :])

    # Force every activation onto the one table set holding Exp+Ln+Square
    import types
    import bass_rust as _bass_rust
    from concourse.hw_specs import get_activation_tables
    import concourse.mybir as mybir2

    def _act_tables_one_set(self):
        has_activation = any(
            isinstance(i, mybir2.InstActivation)
            for b in self.main_func.blocks
            for i in b.instructions
        )
        if not has_activation:
            return
        tables = get_activation_tables(self.m.arch)
        pref = "natural_log_exp_and_others"
        mod = [(k, (v if k == pref else set())) for k, v in tables.items()]
        _bass_rust.insert_act_table_loads(self, mod)

    nc.insert_act_table_loads = types.MethodType(_act_tables_one_set, nc)

    nc.compile()
    return nc


# ----------------------------------------------------------------------------
# Host-side input assembly per core
# ----------------------------------------------------------------------------

def _make_in_maps(feat, batch_ids, conv_w, conv_b, centroids, fc_w, fc_b):
    core_feat, T, n_pad = _plan(feat, batch_ids)

    wt = np.ascontiguousarray(conv_w.T).astype(BF16)                # [256, 64]
    erep = np.exp(conv_b.astype(np.float32) - np.float32(conv_b.max())
                  + np.float32(SHIFT))
    erep_rep = np.broadcast_to(
        np.tile(erep.astype(BF16), G)[None, :], (P, G * K)).copy()  # [128, G*K]
    cent = np.concatenate([centroids, centroids], 0).astype(BF16)   # [128, 256]
    corr_all = np.zeros((B * K, 2), np.float32)
    for b in range(B):
        corr_all[b * K:(b + 1) * K] = _pad_correction(conv_w, conv_b, n_pad[b])
    ident = np.eye(P, dtype=np.float32).astype(BF16)
    sel = np.zeros((P, B), np.float32)
    for gq in range(4):
        for b in range(B):
            sel[32 * gq + b, b] = 1.0

    OSL = OUT // NCORES
    in_maps = []
    for i in range(NCORES):
        cf = core_feat[i]
        featN = np.ascontiguousarray(
            cf.reshape(T, P, C).transpose(1, 0, 2)).astype(BF16)
        featT = np.ascontiguousarray(cf.T).astype(BF16)
        # fc slice, negated, chunk-swizzled: chunk j=(h,k) covers
        # kc = k*256 + h*128 + p  -> fwt[p, j*128+o] = -fc_w[o_base+o, kc]
        fsl = -fc_w[i * OSL:(i + 1) * OSL]                          # [128, 16384]
        f4 = fsl.reshape(OSL, K, 2, P)                              # [o, k, h, p]
        fsw = np.ascontiguousarray(
            f4.transpose(3, 2, 1, 0).reshape(P, 2, K, OSL)          # [p, h, k, o]
             .transpose(0, 2, 1, 3)                                 # [p, k, h, o]
        )
        # chunk order j: j%2 = h, j//2 = k  -> linear layout [p, (k h) o]? no:
        # device uses fwt_sb[:, j*OSL:(j+1)*OSL] with j = (h,k): h=j%2,k=j//2
        # so layout must be [p, k, h, o] flattened over (k, h, o)
        fsw = fsw.reshape(P, K * C).astype(BF16)
        fbb = np.broadcast_to(fc_b[i * OSL:(i + 1) * OSL].astype(np.float32),
                              (B, OSL)).copy()
        in_maps.append({
            "featN": featN,
            "featT": featT,
            "wt": wt,
            "erep": erep_rep,
            "cent": cent,
            "corr": corr_all,
            "fwt": fsw,
            "fbb": fbb,
            "ident": ident,
            "sel": sel,
        })
    return in_maps, T


def _ensure_profile_hook():
    import sys
    import types
    try:
        from antenv.axon_hooks import get_axon_ntff_profile_hook  # noqa: F401
        return True
    except ImportError:
        pass
    try:
        from trn_agent_boot.trn_boot import _ntff_profile_via_ctypes
        hook = _ntff_profile_via_ctypes("/opt/axon/libaxon_pjrt.so")
        if hook is None:
            return False
        mod = types.ModuleType("antenv.axon_hooks")
        mod._hook = hook
        mod.get_axon_ntff_profile_hook = lambda: mod._hook
        mod.set_axon_ntff_profile_hook = lambda h: setattr(mod, "_hook", h)
        import antenv
        antenv.axon_hooks = mod
        sys.modules["antenv.axon_hooks"] = mod
        return True
    except Exception:
        return False


def kernel(feat, batch_ids, centroids, conv_w, conv_b, fc_w, fc_b, batch_size):
    from concourse.bass_utils import run_bass_kernel_spmd

    feat = np.asarray(feat, dtype=np.float32)
    batch_ids = np.asarray(batch_ids, dtype=np.int32)
    centroids = np.asarray(centroids, dtype=np.float32)
    conv_w = np.asarray(conv_w, dtype=np.float32)
    conv_b = np.asarray(conv_b, dtype=np.float32)
    fc_w = np.asarray(fc_w, dtype=np.float32)
    fc_b = np.asarray(fc_b, dtype=np.float32)

    assert conv_b.max() - conv_b.min() < 125.0, "conv_b spread too wide for SHIFT"

    in_maps, T = _make_in_maps(
        feat, batch_ids, conv_w, conv_b, centroids, fc_w, fc_b)

    if T not in _compiled_cache:
        _compiled_cache[T] = _build_nc(T)
    nc = _compiled_cache[T]

    global LAST_RESULT
    do_trace = PROFILE and _ensure_profile_hook()
    import os as _os
    _tc = _os.environ.get("TRACE_CORE")
    _kw = {"trace_cores": [int(_tc)]} if _tc else {}
    res = run_bass_kernel_spmd(
        nc, in_maps, core_ids=list(range(NCORES)), trace=do_trace, **_kw)
    LAST_RESULT = res
    return np.asarray(res.results[0]["out"], dtype=np.float32)



# revision 2
# speedup vs baseline: 1.5494x; 1.5494x over previous
"""NetVLAD Trainium2 kernel (8 NeuronCores, batch-per-core sharding).

Strategy (v2):
  - Host: stable-sort points by batch_id; core i owns batch i entirely,
    padded to T*128 rows (shared T; pads are e0 unit vectors, corrected
    exactly post-aggregation). Rows are L2-normalized on host during the
    bf16 repack, so the device logits PSUM is x_hat @ w directly and the
    whole ssq/r chain disappears. feat ships twice in bf16 (natural with
    a ones column for the S-sum, + transposed) = same bytes as one fp32.
  - Device main loop (groups of G=16 tiles of 128 points):
      logits: 2 bf16 matmuls per tile into shared PSUM banks (8/bank)
      negm = -rowmax per bank (one batched DVE reduce from PSUM)
      arg  = (psum + 35) - max  (one DVE STT per bank, bf16 out)
      e1   = exp(arg)           (ONE batched ACT exp per bank)
      e2   = e1 * E_rep on GpSimd (E[k] = exp(conv_b[k] - max conv_b + 35))
      Z    = rowsum(e2) (DVE), rz = 1/Z
      soft2 = bf16(e2 * rz) in one batched GpSimd op (stride-0 broadcast)
      agg[:,0:257] += soft2^T @ [x_hat | 1]  (ONE matmul per tile; the
      ones column yields the S sums in col 256)
    The agg matmuls for group g are issued one group late so the PE's
    in-order queue never stalls on the softmax chain.
  - Half-1 agg is AllGathered (fp16) mid-loop, fully hidden; only the
    half-2 AllGather is exposed at loop end.  Every core then combines,
    pad-corrects, builds negated VLAD (S*c - A), intra-normalizes all 8
    batches, transposes via PE into FC operand layout, computes its
    128-col FC output slice (negated fc_w), AllGathers the [8,129]
    slices+partial norms, and applies the final l2norm.
"""

import numpy as np
import ml_dtypes

BF16 = ml_dtypes.bfloat16

N, C, K, B, OUT = 200000, 256, 64, 8, 1024
NCORES = 8
P = 128
G = 16                # tiles per group
GB = 8                # tiles per PSUM logits bank
# exp shift, split between the arg bias (+35) and the host constant E (+35):
# keeps e1/e2 in normal bf16 range for conv_b spread <= ~125
SHIFT = 35.0

_compiled_cache = {}
PROFILE = False
LAST_RESULT = None


# ----------------------------------------------------------------------------
# Host-side planning
# ----------------------------------------------------------------------------

def _plan(feat, batch_ids):
    """Sort by batch; core i gets batch i (rows pre-normalized) padded to
    T*128 rows (shared T)."""
    order = np.argsort(batch_ids, kind="stable")
    feat_s = feat[order]
    nrm = np.sqrt(np.einsum("nc,nc->n", feat_s, feat_s, dtype=np.float64))
    nrm = np.maximum(nrm, 1e-12).astype(np.float32)
    feat_s = feat_s * (1.0 / nrm)[:, None]
    counts = np.bincount(batch_ids, minlength=B)
    T = int(np.ceil(counts.max() / P))
    n_pad = [T * P - int(c) for c in counts]

    pad_row = np.zeros((C,), np.float32)
    pad_row[0] = 1.0

    core_feat = []
    off = 0
    for b in range(B):
        nb = int(counts[b])
        fb = feat_s[off:off + nb]
        off += nb
        if n_pad[b]:
            fb = np.concatenate([fb, np.broadcast_to(pad_row, (n_pad[b], C))], 0)
        core_feat.append(fb)
    return core_feat, T, n_pad


def _pad_correction(conv_w, conv_b, n_pad):
    """Exact contribution of one e0 pad row through the device pipeline."""
    w_bf = conv_w.astype(BF16)
    raw = w_bf[:, 0].astype(np.float32)          # x_hat=e0 -> logits psum
    m = raw.max()
    E = np.exp(conv_b.astype(np.float32) - np.float32(conv_b.max())
               + np.float32(SHIFT)).astype(BF16)
    arg = ((raw + np.float32(SHIFT)) - m).astype(BF16)
    e1 = np.exp(arg.astype(np.float32)).astype(BF16)
    e2 = (e1.astype(np.float32) * E.astype(np.float32)).astype(BF16)
    Z = e2.astype(np.float32).sum()
    rz = np.float32(1.0) / Z
    soft2 = (e2.astype(np.float32) * rz).astype(BF16).astype(np.float32)
    # one pad row contributes soft2[k] at col0 (x_hat=e0) and col C (ones)
    corr = np.zeros((K, 2), np.float32)
    corr[:, 0] = n_pad * soft2
    corr[:, 1] = n_pad * soft2
    return corr


# ----------------------------------------------------------------------------
# Device program
# ----------------------------------------------------------------------------

def _build_nc(T):
    import concourse.bass as bass
    import concourse.bacc as bacc
    import concourse.mybir as mybir
    from concourse import tile

    dt = mybir.dt
    AF = mybir.ActivationFunctionType
    ALU = mybir.AluOpType


    NP = T * P
    OSL = OUT // NCORES  # 128 output cols per core
    C1 = C + 1


    nc = bacc.Bacc(
        "TRN2", target_bir_lowering=False, debug=False, num_devices=NCORES
    )

    # --- I/O ---
    featN_d = nc.dram_tensor("featN", [P, T, C1], dt.bfloat16, kind="ExternalInput").ap()
    featT_d = nc.dram_tensor("featT", [C, NP], dt.bfloat16, kind="ExternalInput").ap()
    wt_d = nc.dram_tensor("wt", [C, K], dt.bfloat16, kind="ExternalInput").ap()
    erep_d = nc.dram_tensor("erep", [P, G * K], dt.bfloat16, kind="ExternalInput").ap()
    cent_d = nc.dram_tensor("cent", [P, C], dt.bfloat16, kind="ExternalInput").ap()
    corr_d = nc.dram_tensor("corr", [B * K, 2], dt.float32, kind="ExternalInput").ap()
    fwt_d = nc.dram_tensor("fwt", [P, K * C], dt.bfloat16, kind="ExternalInput").ap()
    fbb_d = nc.dram_tensor("fbb", [B, OSL], dt.float32, kind="ExternalInput").ap()
    ident_d = nc.dram_tensor("ident", [P, P], dt.bfloat16, kind="ExternalInput").ap()
    sel_d = nc.dram_tensor("sel", [P, B], dt.float32, kind="ExternalInput").ap()
    out_d = nc.dram_tensor("out", [B, OUT], dt.float32, kind="ExternalOutput").ap()

    featT_v = featT_d.rearrange("(h p) n -> p h n", h=2)

    with tile.TileContext(nc) as tc:
        with (
            tc.tile_pool(name="const", bufs=1) as cpool,
            tc.tile_pool(name="dram", bufs=1, space="DRAM") as dram,
        ):
            wt_sb = cpool.tile([P, 2, K], dt.bfloat16, name="wt_sb")
            for h in range(2):
                nc.sync.dma_start(out=wt_sb[:, h, :], in_=wt_d[h * P:(h + 1) * P, :])
            erep_sb = cpool.tile([P, G * K], dt.bfloat16, name="erep_sb")
            nc.sync.dma_start(out=erep_sb[:, :], in_=erep_d[:, :])
            # tail-only constants: tiles declared here, DMAs deferred into the
            # loop so the first feat groups win the DMA queues
            cent_sb = cpool.tile([P, C], dt.bfloat16, name="cent_sb")
            corr_sb = cpool.tile([P, 4, 2], dt.float32, name="corr_sb")
            avA = cpool.tile([P, 4, C1], dt.float16, name="avA")
            ident_sb = cpool.tile([P, P], dt.bfloat16, name="ident_sb")
            fbb_sb = cpool.tile([B, OSL], dt.float32, name="fbb_sb")
            sel_sb = cpool.tile([P, B], dt.float32, name="sel_sb")
            fwt_sb = cpool.tile([P, K * C], dt.bfloat16, name="fwt_sb")

            def _load_tail_consts():
                nc.sync.dma_start(out=cent_sb[:, :], in_=cent_d[:, :])
                nc.sync.dma_start(
                    out=corr_sb[:, :, :],
                    in_=corr_d.rearrange("(q p) c -> p q c", q=4))
                nc.sync.dma_start(out=ident_sb[:, :], in_=ident_d[:, :])
                nc.sync.dma_start(out=fbb_sb[:, :], in_=fbb_d[:, :])
                nc.sync.dma_start(out=sel_sb[:, :], in_=sel_d[:, :])

            def _load_fwt_chunk(q):
                qs = K * C // 8
                eng = nc.sync if q % 2 == 0 else nc.scalar
                eng.dma_start(out=fwt_sb[:, q * qs:(q + 1) * qs],
                              in_=fwt_d[:, q * qs:(q + 1) * qs])

            # ---------------- main point loop ----------------
            with (
                tc.tile_pool(name="aggp", bufs=1, space="PSUM") as aggp,
                tc.tile_pool(name="psl", bufs=2, space="PSUM") as pslp,
                tc.tile_pool(name="feed", bufs=4) as fepool,
                tc.tile_pool(name="grp", bufs=3) as gpool,
            ):
                aggA = aggp.tile([K, C1], dt.float32, name="aggA")
                aggB = aggp.tile([K, C1], dt.float32, name="aggB")
                NGRP = (T + G - 1) // G
                HALF = (NGRP // 2) * G  # first tile of bank B

                ag_a_in = dram.tile([K, C1], dt.float16, name="ag_a_in")
                ag_a_out = dram.tile([NCORES * K, C1], dt.float16,
                                     name="ag_a_out")
                ag_b_in = dram.tile([K, C1], dt.float16, name="ag_b_in")
                ag_b_out = dram.tile([NCORES * K, C1], dt.float16,
                                     name="ag_b_out")
                evA = cpool.tile([K, C1], dt.float16, name="evA")

                def do_agg(t0, g_size, featN_g, soft2_g):
                    # aggregation matmuls for tiles [t0, t0+g_size), one
                    # matmul per tile (rhs includes the ones column)
                    for g in range(g_size):
                        tt = t0 + g
                        agg = aggA if tt < HALF else aggB
                        st = (tt == 0 or tt == HALF)
                        sp = (tt == HALF - 1 or tt == T - 1)
                        nc.tensor.matmul(
                            agg[:, :],
                            lhsT=soft2_g[:, g, :],
                            rhs=featN_g[:, g, :],
                            start=st, stop=sp,
                        )

                def do_ag_a():
                    # bank A complete: evac + AllGather (hidden by loop)
                    nc.scalar.copy(evA[:, :], aggA[:, :])
                    nc.sync.dma_start(out=ag_a_in[:, :], in_=evA[:, :])
                    nc.gpsimd.collective_compute(
                        "AllGather",
                        ALU.bypass,
                        replica_groups=[list(range(NCORES))],
                        ins=[ag_a_in[:, :]],
                        outs=[ag_a_out[:, :]],
                    )

                t = 0
                gi = 0
                prev = None  # (t0, g_size, featN_g, soft2_g) pending agg
                while t < T:
                    if gi == 2:
                        _load_tail_consts()
                    if 2 <= gi < 10:
                        _load_fwt_chunk(gi - 2)
                    gi += 1
                    g_size = min(G, T - t)
                    nbank = (g_size + GB - 1) // GB
                    featT_g = fepool.tile([P, 2, G * P], dt.bfloat16, name="featT_g")
                    featN_g = fepool.tile([P, G, C1], dt.bfloat16, name="featN_g")
                    arg_g = gpool.tile([P, G * K], dt.bfloat16, name="arg_g")
                    e1_g = gpool.tile([P, G * K], dt.bfloat16, name="e1_g")
                    e2_g = gpool.tile([P, G * K], dt.bfloat16, name="e2_g")
                    soft2_g = gpool.tile([P, G, K], dt.bfloat16, name="soft2_g")
                    negm_g = gpool.tile([P, G], dt.float32, name="negm_g")
                    z_g = gpool.tile([P, G], dt.float32, name="z_g")
                    rz_g = gpool.tile([P, G], dt.float32, name="rz_g")

                    # one DMA each: featT slab + featN slab
                    nc.sync.dma_start(
                        out=featT_g[:, :, 0:g_size * P],
                        in_=featT_v[:, :, t * P:(t + g_size) * P],
                    )
                    nc.scalar.dma_start(
                        out=featN_g[:, 0:g_size, :],
                        in_=featN_d[:, t:t + g_size, :],
                    )

                    # logits matmuls into shared PSUM banks (GB tiles each)
                    banks = [pslp.tile([P, GB * K], dt.float32, name=f"bank{i}",
                                       tag=f"bank{i}") for i in range(nbank)]
                    for g in range(g_size):
                        bk, sl = banks[g // GB], (g % GB) * K
                        nc.tensor.matmul(
                            bk[:, sl:sl + K],
                            lhsT=featT_g[:, 0, g * P:(g + 1) * P],
                            rhs=wt_sb[:, 0, :],
                            start=True, stop=False,
                        )
                        nc.tensor.matmul(
                            bk[:, sl:sl + K],
                            lhsT=featT_g[:, 1, g * P:(g + 1) * P],
                            rhs=wt_sb[:, 1, :],
                            start=False, stop=True,
                        )

                    # per bank: negated rowmax, arg = (psum+35)-max, exp
                    for i in range(nbank):
                        lo = i * GB
                        n_in = min(GB, g_size - lo)
                        bk3 = banks[i].rearrange("p (g k) -> p g k", k=K)
                        nc.vector.tensor_reduce(
                            out=negm_g[:, lo:lo + n_in],
                            in_=bk3[:, 0:n_in, :],
                            axis=mybir.AxisListType.X,
                            op=ALU.max,
                            negate=True,
                        )
                        nc.vector.scalar_tensor_tensor(
                            out=arg_g.rearrange("p (g k) -> p g k", k=K)[
                                :, lo:lo + n_in, :],
                            in0=bk3[:, 0:n_in, :],
                            scalar=SHIFT,
                            in1=negm_g[:, lo:lo + n_in]
                                .rearrange("p g -> p g ()")
                                .broadcast_to([P, n_in, K]),
                            op0=ALU.add,
                            op1=ALU.add,
                        )
                        nc.scalar.activation(
                            e1_g[:, lo * K:(lo + n_in) * K],
                            arg_g[:, lo * K:(lo + n_in) * K],
                            AF.Exp,
                        )
                    # e2 = e1 * E_rep (gpsimd TT), Z = rowsum (DVE)
                    nc.gpsimd.tensor_tensor(
                        out=e2_g[:, 0:g_size * K],
                        in0=e1_g[:, 0:g_size * K],
                        in1=erep_sb[:, 0:g_size * K],
                        op=ALU.mult,
                    )
                    nc.vector.tensor_reduce(
                        out=z_g[:, 0:g_size],
                        in_=e2_g.rearrange("p (g k) -> p g k", k=K)[:, 0:g_size, :],
                        axis=mybir.AxisListType.X,
                        op=ALU.add,
                    )
                    nc.vector.reciprocal(rz_g[:, 0:g_size], z_g[:, 0:g_size])
                    # soft2 = e2 * rz (one batched GpSimd op, rz broadcast)
                    nc.gpsimd.tensor_tensor(
                        out=soft2_g[:, 0:g_size, :],
                        in0=e2_g.rearrange("p (g k) -> p g k", k=K)[:, 0:g_size, :],
                        in1=rz_g[:, 0:g_size].rearrange("p g -> p g ()")
                            .broadcast_to([P, g_size, K]),
                        op=ALU.mult,
                    )
                    # aggregation for the PREVIOUS group (keeps the PE's
                    # in-order queue free of the softmax-chain dependency)
                    if prev is not None:
                        do_agg(*prev)
                        if prev[0] + prev[1] == HALF:
                            do_ag_a()
                    prev = (t, g_size, featN_g, soft2_g)
                    t += g_size
                do_agg(*prev)
                if prev[0] + prev[1] == HALF:
                    do_ag_a()

            # ---------------- tail: vlad, AG, fc, AG, norm ----------------
            with (
                tc.tile_pool(name="fin", bufs=1) as fpool,
                tc.tile_pool(name="fps", bufs=2, space="PSUM") as fpsum,
                tc.tile_pool(name="fcp", bufs=1, space="PSUM") as fcps,
            ):
                # evac bank B, AllGather it
                evB = fpool.tile([K, C1], dt.float16, name="evB")
                nc.scalar.copy(evB[:, :], aggB[:, :])
                nc.sync.dma_start(out=ag_b_in[:, :], in_=evB[:, :])
                nc.gpsimd.collective_compute(
                    "AllGather",
                    ALU.bypass,
                    replica_groups=[list(range(NCORES))],
                    ins=[ag_b_in[:, :]],
                    outs=[ag_b_out[:, :]],
                )
                # combine halves, pad-correct, vlad, intra-normalize:
                # 4 tiles of 128 rows, tile q = batches {2q, 2q+1}
                # avA stages during the AG-b wait (depends only on AG-a)
                nc.sync.dma_start(
                    out=avA[:, :, :],
                    in_=ag_a_out.rearrange("(q p) c -> p q c", q=4))
                avB = fpool.tile([P, 4, C1], dt.float16, name="avB")
                nc.sync.dma_start(
                    out=avB[:, :, :],
                    in_=ag_b_out.rearrange("(q p) c -> p q c", q=4))
                ssv = fpool.tile([P, 4], dt.float32, name="ssv")
                lnv = fpool.tile([P, 4], dt.float32, name="lnv")
                rnv = fpool.tile([P, 4], dt.float32, name="rnv")
                vT_all = fpool.tile([P, 2, B, K], dt.bfloat16, name="vT_all")
                avS = fpool.tile([P, 4, C1], dt.float32, name="avS")
                nc.vector.tensor_tensor(
                    out=avS[:, :, :], in0=avA[:, :, :], in1=avB[:, :, :],
                    op=ALU.add)
                # pad correction touches only cols 0 and C
                nc.vector.tensor_tensor(
                    out=avS[:, :, 0], in0=avS[:, :, 0], in1=corr_sb[:, :, 0],
                    op=ALU.subtract)
                nc.vector.tensor_tensor(
                    out=avS[:, :, C], in0=avS[:, :, C], in1=corr_sb[:, :, 1],
                    op=ALU.subtract)
                nvq = []
                for q in range(4):
                    nv = fpool.tile([P, C], dt.float32, name="nv", tag="nv",
                                    bufs=4)
                    nvq.append(nv)
                    nc.vector.scalar_tensor_tensor(
                        out=nv[:, :], in0=cent_sb[:, :],
                        scalar=avS[:, q, C:C + 1], in1=avS[:, q, 0:C],
                        op0=ALU.mult, op1=ALU.subtract)
                    nvs = fpool.tile([P, C], dt.float32, name="nvs", tag="nvs",
                                     bufs=2)
                    if q % 2 == 1:
                        nc.scalar.activation(
                            nvs[:, :], nv[:, :], AF.Square,
                            accum_out=ssv[:, q:q + 1])
                    else:
                        nc.vector.scalar_tensor_tensor(
                            out=nvs[:, :], in0=nv[:, :], scalar=1.0, in1=nv[:, :],
                            op0=ALU.mult, op1=ALU.mult, accum_out=ssv[:, q:q + 1])
                nc.vector.tensor_scalar_max(ssv[:, :], ssv[:, :], 1e-24)
                nc.scalar.activation(lnv[:, :], ssv[:, :], AF.Ln)
                nc.scalar.activation(rnv[:, :], lnv[:, :], AF.Exp, scale=-0.5)
                ptb = [fpsum.tile([P, 4 * P], dt.bfloat16, name=f"ptb{h}",
                                  bufs=1) for h in range(2)]
                for q in range(4):
                    vbf = fpool.tile([P, C], dt.bfloat16, name="vbf",
                                     tag="vbf", bufs=2)
                    nc.vector.tensor_scalar(
                        out=vbf[:, :], in0=nvq[q][:, :],
                        scalar1=rnv[:, q:q + 1], scalar2=None, op0=ALU.mult)
                    for h in range(2):
                        nc.tensor.transpose(
                            ptb[h][:, q * P:(q + 1) * P],
                            vbf[:, h * P:(h + 1) * P],
                            ident_sb[:, :])
                # contiguous evacs; bank col layout is already 64*b + k, so
                # the FC reads lhsT with a single stride-64 AP — no permute
                for h in range(2):
                    nc.vector.tensor_copy(vT_all[:, h, :, :], ptb[h][:, :])

                # FC: out[8b, 128o] in 4 concurrent col-groups, separate banks
                # chunk j=(h,k): lhsT = vT_all[:, :, h, (j k)] -> [128, 4, 2]
                fcpg = [fcps.tile([P, OSL], dt.float32, name=f"fcp{gq}", bufs=1)
                        for gq in range(4)]
                NCH = K * C // P  # 128
                for j in range(NCH):
                    grp = j % 4
                    h, k = j % 2, j // 2
                    nc.tensor.matmul(
                        fcpg[grp][32 * grp:32 * grp + B, :],
                        lhsT=vT_all[:, h, :, k],
                        rhs=fwt_sb[:, j * OSL:(j + 1) * OSL],
                        start=(j < 4), stop=(j >= NCH - 4),
                        tile_position=(0, 32 * grp),
                        skip_group_check=True,
                    )
                sb4 = fpool.tile([P, OSL], dt.float32, name="sb4")
                nc.vector.memset(sb4[:, :], 0.0)
                for gq in range(4):
                    nc.scalar.copy(
                        sb4[32 * gq:32 * gq + B, :],
                        fcpg[gq][32 * gq:32 * gq + B, :])
                fcsum = fcps.tile([P, OSL], dt.float32, name="fcsum", bufs=1)
                nc.tensor.matmul(
                    fcsum[0:B, :], lhsT=sel_sb[:, :], rhs=sb4[:, :],
                    start=True, stop=True, skip_group_check=True,
                )
                fo = fpool.tile([B, OSL], dt.float32, name="fo")
                nc.vector.tensor_tensor(
                    out=fo[:, :], in0=fcsum[0:B, :], in1=fbb_sb[:, :],
                    op=ALU.add)

                # AllGather the [8, 128] slices + per-core partial sumsq
                fop = fpool.tile([B, OSL + 1], dt.float32, name="fop")
                nc.vector.scalar_tensor_tensor(
                    out=fop[:, 0:OSL], in0=fo[:, :], scalar=1.0,
                    in1=fo[:, :], op0=ALU.mult, op1=ALU.mult,
                    accum_out=fop[:, OSL:OSL + 1])
                nc.vector.tensor_copy(fop[:, 0:OSL], fo[:, :])
                ag_in = dram.tile([B, OSL + 1], dt.float32, name="ag_in")
                ag_out = dram.tile([NCORES * B, OSL + 1], dt.float32, name="ag_out")
                nc.sync.dma_start(out=ag_in[:, :], in_=fop[:, :])
                nc.gpsimd.collective_compute(
                    "AllGather",
                    ALU.bypass,
                    replica_groups=[list(range(NCORES))],
                    ins=[ag_in[:, :]],
                    outs=[ag_out[:, :]],
                )
                fin = fpool.tile([B, OUT], dt.float32, name="fin")
                agv = ag_out.rearrange("(c b) o -> b c o", b=B)
                nc.sync.dma_start(
                    out=fin.rearrange("b (c o) -> b c o", c=NCORES),
                    in_=agv[:, :, 0:OSL],
                )
                ssfp = fpool.tile([B, NCORES], dt.float32, name="ssfp")
                nc.sync.dma_start(out=ssfp[:, :], in_=agv[:, :, OSL])
                ssf = fpool.tile([B, 1], dt.float32, name="ssf")
                lnf = fpool.tile([B, 1], dt.float32, name="lnf")
                rnf = fpool.tile([B, 1], dt.float32, name="rnf")
                nc.vector.tensor_reduce(
                    out=ssf[:, :], in_=ssfp[:, :],
                    axis=mybir.AxisListType.X, op=ALU.add)
                nc.vector.tensor_scalar_max(ssf[:, :], ssf[:, :], 1e-24)
                nc.scalar.activation(lnf[:, :], ssf[:, :], AF.Ln)
                nc.scalar.activation(rnf[:, :], lnf[:, :], AF.Exp, scale=-0.5)
                fout = fpool.tile([B, OUT], dt.float32, name="fout")
                nc.vector.tensor_scalar(
                    out=fout[:, :], in0=fin[:, :],
                    scalar1=rnf[:, 0:1], scalar2=None, op0=ALU.mult)
                nc.sync.dma_start(out=out_d[:, :], in_=fout[:, :])

    # Force every activation onto the one table set holding Exp+Ln+Square
    import types
    import bass_rust as _bass_rust
    from concourse.hw_specs import get_activation_tables
    import concourse.mybir as mybir2

    def _act_tables_one_set(self):
        has_activation = any(
            isinstance(i, mybir2.InstActivation)
            for b in self.main_func.blocks
            for i in b.instructions
        )
        if not has_activation:
            return
        tables = get_activation_tables(self.m.arch)
        pref = "natural_log_exp_and_others"
        mod = [(k, (v if k == pref else set())) for k, v in tables.items()]
        _bass_rust.insert_act_table_loads(self, mod)

    nc.insert_act_table_loads = types.MethodType(_act_tables_one_set, nc)

    nc.compile()
    return nc


# ----------------------------------------------------------------------------
# Host-side input assembly per core
# ----------------------------------------------------------------------------

def _make_in_maps(feat, batch_ids, conv_w, conv_b, centroids, fc_w, fc_b):
    core_feat, T, n_pad = _plan(feat, batch_ids)

    wt = np.ascontiguousarray(conv_w.T).astype(BF16)                # [256, 64]
    erep = np.exp(conv_b.astype(np.float32) - np.float32(conv_b.max())
                  + np.float32(SHIFT))
    erep_rep = np.broadcast_to(
        np.tile(erep.astype(BF16), G)[None, :], (P, G * K)).copy()  # [128, G*K]
    cent = np.concatenate([centroids, centroids], 0).astype(BF16)   # [128, 256]
    corr_all = np.zeros((B * K, 2), np.float32)
    for b in range(B):
        corr_all[b * K:(b + 1) * K] = _pad_correction(conv_w, conv_b, n_pad[b])
    ident = np.eye(P, dtype=np.float32).astype(BF16)
    sel = np.zeros((P, B), np.float32)
    for gq in range(4):
        for b in range(B):
            sel[32 * gq + b, b] = 1.0

    OSL = OUT // NCORES
    in_maps = []
    for i in range(NCORES):
        cf = core_feat[i]
        featN = np.empty((P, T, C + 1), dtype=BF16)
        featN[:, :, 0:C] = cf.reshape(T, P, C).transpose(1, 0, 2).astype(BF16)
        featN[:, :, C] = BF16(1.0)
        featT = np.ascontiguousarray(cf.T).astype(BF16)
        # fc slice, negated, chunk-swizzled: chunk j=(h,k) covers
        # kc = k*256 + h*128 + p  -> fwt[p, j*128+o] = -fc_w[o_base+o, kc]
        fsl = -fc_w[i * OSL:(i + 1) * OSL]                          # [128, 16384]
        f4 = fsl.reshape(OSL, K, 2, P)                              # [o, k, h, p]
        fsw = np.ascontiguousarray(
            f4.transpose(3, 2, 1, 0).reshape(P, 2, K, OSL)          # [p, h, k, o]
             .transpose(0, 2, 1, 3)                                 # [p, k, h, o]
        )
        # chunk order j: j%2 = h, j//2 = k -> layout [p, (k h) o]
        fsw = fsw.reshape(P, K * C).astype(BF16)
        fbb = np.broadcast_to(fc_b[i * OSL:(i + 1) * OSL].astype(np.float32),
                              (B, OSL)).copy()
        in_maps.append({
            "featN": featN,
            "featT": featT,
            "wt": wt,
            "erep": erep_rep,
            "cent": cent,
            "corr": corr_all,
            "fwt": fsw,
            "fbb": fbb,
            "ident": ident,
            "sel": sel,
        })
    return in_maps, T


def _ensure_profile_hook():
    import sys
    import types
    try:
        from antenv.axon_hooks import get_axon_ntff_profile_hook  # noqa: F401
        return True
    except ImportError:
        pass
    try:
        from trn_agent_boot.trn_boot import _ntff_profile_via_ctypes
        hook = _ntff_profile_via_ctypes("/opt/axon/libaxon_pjrt.so")
        if hook is None:
            return False
        mod = types.ModuleType("antenv.axon_hooks")
        mod._hook = hook
        mod.get_axon_ntff_profile_hook = lambda: mod._hook
        mod.set_axon_ntff_profile_hook = lambda h: setattr(mod, "_hook", h)
        import antenv
        antenv.axon_hooks = mod
        sys.modules["antenv.axon_hooks"] = mod
        return True
    except Exception:
        return False


def kernel(feat, batch_ids, centroids, conv_w, conv_b, fc_w, fc_b, batch_size):
    from concourse.bass_utils import run_bass_kernel_spmd

    feat = np.asarray(feat, dtype=np.float32)
    batch_ids = np.asarray(batch_ids, dtype=np.int32)
    centroids = np.asarray(centroids, dtype=np.float32)
    conv_w = np.asarray(conv_w, dtype=np.float32)
    conv_b = np.asarray(conv_b, dtype=np.float32)
    fc_w = np.asarray(fc_w, dtype=np.float32)
    fc_b = np.asarray(fc_b, dtype=np.float32)

    assert conv_b.max() - conv_b.min() < 125.0, "conv_b spread too wide for SHIFT"

    in_maps, T = _make_in_maps(
        feat, batch_ids, conv_w, conv_b, centroids, fc_w, fc_b)

    if T not in _compiled_cache:
        _compiled_cache[T] = _build_nc(T)
    nc = _compiled_cache[T]

    global LAST_RESULT
    do_trace = PROFILE and _ensure_profile_hook()
    import os as _os
    _tc = _os.environ.get("TRACE_CORE")
    _kw = {"trace_cores": [int(_tc)]} if _tc else {}
    res = run_bass_kernel_spmd(
        nc, in_maps, core_ids=list(range(NCORES)), trace=do_trace, **_kw)
    LAST_RESULT = res
    return np.asarray(res.results[0]["out"], dtype=np.float32)


# revision 6
# speedup vs baseline: 1.7640x; 1.1385x over previous
"""NetVLAD Trainium2 kernel (8 NeuronCores, batch-per-core sharding).

Strategy (v2):
  - Host: stable-sort points by batch_id; core i owns batch i entirely,
    padded to T*128 rows (shared T; pads are e0 unit vectors, corrected
    exactly post-aggregation). Rows are L2-normalized on host during the
    bf16 repack, so the device logits PSUM is x_hat @ w directly and the
    whole ssq/r chain disappears. feat ships twice in bf16 (natural with
    a ones column for the S-sum, + transposed) = same bytes as one fp32.
  - Device main loop (groups of G=16 tiles of 128 points):
      logits: 2 bf16 matmuls per tile into shared PSUM banks (8/bank)
      negm = -rowmax per bank (one batched DVE reduce from PSUM)
      arg  = (psum + 35) - max  (one DVE STT per bank, bf16 out)
      e1   = exp(arg)           (ONE batched ACT exp per bank)
      e2   = e1 * E_rep on GpSimd (E[k] = exp(conv_b[k] - max conv_b + 35))
      Z    = rowsum(e2) (DVE), rz = 1/Z
      soft2 = bf16(e2 * rz) in one batched GpSimd op (stride-0 broadcast)
      agg[:,0:257] += soft2^T @ [x_hat | 1]  (ONE matmul per tile; the
      ones column yields the S sums in col 256)
    The agg matmuls for group g are issued one group late so the PE's
    in-order queue never stalls on the softmax chain.
  - Half-1 agg is AllGathered (fp16) mid-loop, fully hidden; only the
    half-2 AllGather is exposed at loop end.  Every core then combines,
    pad-corrects, builds negated VLAD (S*c - A), intra-normalizes all 8
    batches, transposes via PE into FC operand layout, computes its
    128-col FC output slice (negated fc_w), AllGathers the [8,129]
    slices+partial norms, and applies the final l2norm.
"""

import numpy as np
import ml_dtypes

BF16 = ml_dtypes.bfloat16
FP8 = ml_dtypes.float8_e4m3

N, C, K, B, OUT = 200000, 256, 64, 8, 1024
NCORES = 8
P = 128
G = 16                # tiles per group
GB = 8                # tiles per PSUM logits bank
# exp shift, split between the arg bias (+35) and the host constant E (+35):
# keeps e1/e2 in normal bf16 range for conv_b spread <= ~125
SHIFT = 35.0

_compiled_cache = {}
PROFILE = False
LAST_RESULT = None


# ----------------------------------------------------------------------------
# Host-side planning
# ----------------------------------------------------------------------------

def _plan(feat, batch_ids):
    """Sort by batch; core i gets batch i (rows pre-normalized) padded to
    T*128 rows (shared T)."""
    order = np.argsort(batch_ids, kind="stable")
    feat_s = feat[order]
    nrm = np.sqrt(np.einsum("nc,nc->n", feat_s, feat_s, dtype=np.float64))
    nrm = np.maximum(nrm, 1e-12).astype(np.float32)
    feat_s = feat_s * (1.0 / nrm)[:, None]
    counts = np.bincount(batch_ids, minlength=B)
    T = int(np.ceil(counts.max() / P))
    n_pad = [T * P - int(c) for c in counts]

    pad_row = np.zeros((C,), np.float32)
    pad_row[0] = 1.0

    core_feat = []
    off = 0
    for b in range(B):
        nb = int(counts[b])
        fb = feat_s[off:off + nb]
        off += nb
        if n_pad[b]:
            fb = np.concatenate([fb, np.broadcast_to(pad_row, (n_pad[b], C))], 0)
        core_feat.append(fb)
    return core_feat, T, n_pad


def _pad_correction(conv_w, conv_b, n_pad):
    """Exact contribution of one e0 pad row through the device pipeline."""
    w_bf = conv_w.astype(BF16)
    raw = w_bf[:, 0].astype(np.float32)          # x_hat=e0 -> logits psum
    m = raw.max()
    E = np.exp(conv_b.astype(np.float32) - np.float32(conv_b.max())
               + np.float32(SHIFT)).astype(BF16)
    arg = ((raw + np.float32(SHIFT)) - m).astype(BF16)
    e1 = np.exp(arg.astype(np.float32)).astype(BF16)
    e2 = (e1.astype(np.float32) * E.astype(np.float32)).astype(BF16)
    Z = e2.astype(np.float32).sum()
    rz = np.float32(1.0) / Z
    soft2 = (e2.astype(np.float32) * rz).astype(FP8).astype(np.float32)
    # one pad row contributes soft2[k] at col0 (x_hat=e0) and col C (ones)
    corr = np.zeros((K, 2), np.float32)
    corr[:, 0] = n_pad * soft2
    corr[:, 1] = n_pad * soft2
    return corr


# ----------------------------------------------------------------------------
# Device program
# ----------------------------------------------------------------------------

def _build_nc(T):
    import concourse.bass as bass
    import concourse.bacc as bacc
    import concourse.mybir as mybir
    from concourse import tile

    dt = mybir.dt
    AF = mybir.ActivationFunctionType
    ALU = mybir.AluOpType


    NP = T * P
    OSL = OUT // NCORES  # 128 output cols per core
    C1 = C + 1


    nc = bacc.Bacc(
        "TRN2", target_bir_lowering=False, debug=False, num_devices=NCORES
    )

    # --- I/O ---
    featN_d = nc.dram_tensor("featN", [P, T, C1], dt.float8e4, kind="ExternalInput").ap()
    featT_d = nc.dram_tensor("featT", [C, NP], dt.bfloat16, kind="ExternalInput").ap()
    wt_d = nc.dram_tensor("wt", [C, K], dt.bfloat16, kind="ExternalInput").ap()
    erep_d = nc.dram_tensor("erep", [P, G * K], dt.bfloat16, kind="ExternalInput").ap()
    cent_d = nc.dram_tensor("cent", [P, C], dt.bfloat16, kind="ExternalInput").ap()
    corr_d = nc.dram_tensor("corr", [B * K, 2], dt.float32, kind="ExternalInput").ap()
    fwt_d = nc.dram_tensor("fwt", [P, K * C], dt.bfloat16, kind="ExternalInput").ap()
    fbb_d = nc.dram_tensor("fbb", [B, OSL], dt.float32, kind="ExternalInput").ap()
    ident_d = nc.dram_tensor("ident", [P, P], dt.bfloat16, kind="ExternalInput").ap()
    sel_d = nc.dram_tensor("sel", [P, B], dt.float32, kind="ExternalInput").ap()
    out_d = nc.dram_tensor("out", [B, OUT], dt.float32, kind="ExternalOutput").ap()

    featT_v = featT_d.rearrange("(h p) n -> p h n", h=2)

    with tile.TileContext(nc) as tc:
        with (
            tc.tile_pool(name="const", bufs=1) as cpool,
            tc.tile_pool(name="dram", bufs=1, space="DRAM") as dram,
        ):
            wt_sb = cpool.tile([P, 2, K], dt.bfloat16, name="wt_sb")
            for h in range(2):
                nc.sync.dma_start(out=wt_sb[:, h, :], in_=wt_d[h * P:(h + 1) * P, :])
            erep_sb = cpool.tile([P, G * K], dt.bfloat16, name="erep_sb")
            nc.sync.dma_start(out=erep_sb[:, :], in_=erep_d[:, :])
            # tail-only constants: tiles declared here, DMAs deferred into the
            # loop so the first feat groups win the DMA queues
            cent_sb = cpool.tile([P, C], dt.bfloat16, name="cent_sb")
            corr_sb = cpool.tile([P, 4, 2], dt.float32, name="corr_sb")
            avA = cpool.tile([P, 4, C1], dt.float16, name="avA")
            ident_sb = cpool.tile([P, P], dt.bfloat16, name="ident_sb")
            fbb_sb = cpool.tile([B, OSL], dt.float32, name="fbb_sb")
            sel_sb = cpool.tile([P, B], dt.float32, name="sel_sb")
            fwt_sb = cpool.tile([P, K * C], dt.bfloat16, name="fwt_sb")

            def _load_tail_consts():
                nc.sync.dma_start(out=cent_sb[:, :], in_=cent_d[:, :])
                nc.sync.dma_start(
                    out=corr_sb[:, :, :],
                    in_=corr_d.rearrange("(q p) c -> p q c", q=4))
                nc.sync.dma_start(out=ident_sb[:, :], in_=ident_d[:, :])
                nc.sync.dma_start(out=fbb_sb[:, :], in_=fbb_d[:, :])
                nc.sync.dma_start(out=sel_sb[:, :], in_=sel_d[:, :])

            def _load_fwt_chunk(q):
                qs = K * C // 8
                eng = nc.sync if q % 2 == 0 else nc.scalar
                eng.dma_start(out=fwt_sb[:, q * qs:(q + 1) * qs],
                              in_=fwt_d[:, q * qs:(q + 1) * qs])

            # ---------------- main point loop ----------------
            with (
                tc.tile_pool(name="aggp", bufs=1, space="PSUM") as aggp,
                tc.tile_pool(name="psl", bufs=2, space="PSUM") as pslp,
                tc.tile_pool(name="feed", bufs=4) as fepool,
                tc.tile_pool(name="grp", bufs=3) as gpool,
            ):
                aggA = aggp.tile([K, C1], dt.float32, name="aggA")
                aggB = aggp.tile([K, C1], dt.float32, name="aggB")
                NGRP = (T + G - 1) // G
                HALF = (NGRP // 2) * G  # first tile of bank B

                ag_a_in = dram.tile([K, C1], dt.float16, name="ag_a_in")
                ag_a_out = dram.tile([NCORES * K, C1], dt.float16,
                                     name="ag_a_out")
                ag_b_in = dram.tile([K, C1], dt.float16, name="ag_b_in")
                ag_b_out = dram.tile([NCORES * K, C1], dt.float16,
                                     name="ag_b_out")
                evA = cpool.tile([K, C1], dt.float16, name="evA")

                def do_agg(t0, g_size, featN_g, soft2_g):
                    # aggregation matmuls for tiles [t0, t0+g_size), one
                    # matmul per tile (rhs includes the ones column)
                    for g in range(g_size):
                        tt = t0 + g
                        agg = aggA if tt < HALF else aggB
                        st = (tt == 0 or tt == HALF)
                        sp = (tt == HALF - 1 or tt == T - 1)
                        nc.tensor.matmul(
                            agg[:, :],
                            lhsT=soft2_g[:, g, :],
                            rhs=featN_g[:, g, :],
                            start=st, stop=sp,
                        )

                def do_ag_a():
                    # bank A complete: evac + AllGather (hidden by loop)
                    nc.scalar.copy(evA[:, :], aggA[:, :])
                    nc.sync.dma_start(out=ag_a_in[:, :], in_=evA[:, :])
                    nc.gpsimd.collective_compute(
                        "AllGather",
                        ALU.bypass,
                        replica_groups=[list(range(NCORES))],
                        ins=[ag_a_in[:, :]],
                        outs=[ag_a_out[:, :]],
                    )

                t = 0
                gi = 0
                prev = None  # (t0, g_size, featN_g, soft2_g) pending agg
                while t < T:
                    if gi == 2:
                        _load_tail_consts()
                    if 2 <= gi < 10:
                        _load_fwt_chunk(gi - 2)
                    gi += 1
                    g_size = min(G, T - t)
                    nbank = (g_size + GB - 1) // GB
                    featT_g = fepool.tile([P, 2, G * P], dt.bfloat16, name="featT_g")
                    featN_g = fepool.tile([P, G, C1], dt.float8e4, name="featN_g")
                    arg_g = gpool.tile([P, G * K], dt.bfloat16, name="arg_g")
                    e1_g = gpool.tile([P, G * K], dt.bfloat16, name="e1_g")
                    e2_g = gpool.tile([P, G * K], dt.bfloat16, name="e2_g")
                    soft2_g = gpool.tile([P, G, K], dt.float8e4, name="soft2_g")
                    negm_g = gpool.tile([P, G], dt.float32, name="negm_g")
                    z_g = gpool.tile([P, G], dt.float32, name="z_g")
                    rz_g = gpool.tile([P, G], dt.float32, name="rz_g")

                    # one DMA each: featT slab + featN slab
                    nc.sync.dma_start(
                        out=featT_g[:, :, 0:g_size * P],
                        in_=featT_v[:, :, t * P:(t + g_size) * P],
                    )
                    nc.scalar.dma_start(
                        out=featN_g[:, 0:g_size, :],
                        in_=featN_d[:, t:t + g_size, :],
                    )

                    # logits matmuls into shared PSUM banks (GB tiles each)
                    banks = [pslp.tile([P, GB * K], dt.float32, name=f"bank{i}",
                                       tag=f"bank{i}") for i in range(nbank)]
                    for g in range(g_size):
                        bk, sl = banks[g // GB], (g % GB) * K
                        nc.tensor.matmul(
                            bk[:, sl:sl + K],
                            lhsT=featT_g[:, 0, g * P:(g + 1) * P],
                            rhs=wt_sb[:, 0, :],
                            start=True, stop=False,
                        )
                        nc.tensor.matmul(
                            bk[:, sl:sl + K],
                            lhsT=featT_g[:, 1, g * P:(g + 1) * P],
                            rhs=wt_sb[:, 1, :],
                            start=False, stop=True,
                        )

                    # aggregation for the PREVIOUS group (keeps the PE's
                    # in-order queue free of the softmax-chain dependency);
                    # emitted before this group's softmax ops so the AG-A
                    # trigger sits early in the gpsimd queue
                    if prev is not None:
                        do_agg(*prev)
                        if prev[0] + prev[1] == HALF:
                            do_ag_a()
                        prev = None

                    # per bank: negated rowmax, arg = (psum+35)-max, exp
                    for i in range(nbank):
                        lo = i * GB
                        n_in = min(GB, g_size - lo)
                        bk3 = banks[i].rearrange("p (g k) -> p g k", k=K)
                        nc.vector.tensor_reduce(
                            out=negm_g[:, lo:lo + n_in],
                            in_=bk3[:, 0:n_in, :],
                            axis=mybir.AxisListType.X,
                            op=ALU.max,
                            negate=True,
                        )
                        nc.vector.scalar_tensor_tensor(
                            out=arg_g.rearrange("p (g k) -> p g k", k=K)[
                                :, lo:lo + n_in, :],
                            in0=bk3[:, 0:n_in, :],
                            scalar=SHIFT,
                            in1=negm_g[:, lo:lo + n_in]
                                .rearrange("p g -> p g ()")
                                .broadcast_to([P, n_in, K]),
                            op0=ALU.add,
                            op1=ALU.add,
                        )
                        nc.scalar.activation(
                            e1_g[:, lo * K:(lo + n_in) * K],
                            arg_g[:, lo * K:(lo + n_in) * K],
                            AF.Exp,
                        )
                    # e2 = e1 * E_rep (gpsimd TT), Z = rowsum (DVE)
                    nc.gpsimd.tensor_tensor(
                        out=e2_g[:, 0:g_size * K],
                        in0=e1_g[:, 0:g_size * K],
                        in1=erep_sb[:, 0:g_size * K],
                        op=ALU.mult,
                    )
                    nc.vector.tensor_reduce(
                        out=z_g[:, 0:g_size],
                        in_=e2_g.rearrange("p (g k) -> p g k", k=K)[:, 0:g_size, :],
                        axis=mybir.AxisListType.X,
                        op=ALU.add,
                    )
                    nc.vector.reciprocal(rz_g[:, 0:g_size], z_g[:, 0:g_size])
                    # soft2 = e2 * rz (one batched GpSimd op, rz broadcast)
                    nc.gpsimd.tensor_tensor(
                        out=soft2_g[:, 0:g_size, :],
                        in0=e2_g.rearrange("p (g k) -> p g k", k=K)[:, 0:g_size, :],
                        in1=rz_g[:, 0:g_size].rearrange("p g -> p g ()")
                            .broadcast_to([P, g_size, K]),
                        op=ALU.mult,
                    )
                    prev = (t, g_size, featN_g, soft2_g)
                    t += g_size
                do_agg(*prev)
                if prev[0] + prev[1] == HALF:
                    do_ag_a()

            # ---------------- tail: vlad, AG, fc, AG, norm ----------------
            with (
                tc.tile_pool(name="fin", bufs=1) as fpool,
                tc.tile_pool(name="fps", bufs=2, space="PSUM") as fpsum,
                tc.tile_pool(name="fcp", bufs=1, space="PSUM") as fcps,
            ):
                # evac bank B, AllGather it
                evB = fpool.tile([K, C1], dt.float16, name="evB")
                nc.scalar.copy(evB[:, :], aggB[:, :])
                nc.sync.dma_start(out=ag_b_in[:, :], in_=evB[:, :])
                nc.gpsimd.collective_compute(
                    "AllGather",
                    ALU.bypass,
                    replica_groups=[list(range(NCORES))],
                    ins=[ag_b_in[:, :]],
                    outs=[ag_b_out[:, :]],
                )
                # combine halves, pad-correct, vlad, intra-normalize:
                # 4 tiles of 128 rows, tile q = batches {2q, 2q+1}
                # avA stages during the AG-b wait (depends only on AG-a)
                nc.sync.dma_start(
                    out=avA[:, :, :],
                    in_=ag_a_out.rearrange("(q p) c -> p q c", q=4))
                avB = fpool.tile([P, 4, C1], dt.float16, name="avB")
                nc.sync.dma_start(
                    out=avB[:, :, :],
                    in_=ag_b_out.rearrange("(q p) c -> p q c", q=4))
                ssv = fpool.tile([P, 4], dt.float32, name="ssv")
                lnv = fpool.tile([P, 4], dt.float32, name="lnv")
                rnv = fpool.tile([P, 4], dt.float32, name="rnv")
                vT_all = fpool.tile([P, 2, B, K], dt.bfloat16, name="vT_all")
                avS = fpool.tile([P, 4, C1], dt.float32, name="avS")
                nc.vector.tensor_tensor(
                    out=avS[:, :, :], in0=avA[:, :, :], in1=avB[:, :, :],
                    op=ALU.add)
                # pad correction touches only cols 0 and C
                nc.vector.tensor_tensor(
                    out=avS[:, :, 0], in0=avS[:, :, 0], in1=corr_sb[:, :, 0],
                    op=ALU.subtract)
                nc.vector.tensor_tensor(
                    out=avS[:, :, C], in0=avS[:, :, C], in1=corr_sb[:, :, 1],
                    op=ALU.subtract)
                nvq = []
                for q in range(4):
                    nv = fpool.tile([P, C], dt.float32, name="nv", tag="nv",
                                    bufs=4)
                    nvq.append(nv)
                    nc.vector.scalar_tensor_tensor(
                        out=nv[:, :], in0=cent_sb[:, :],
                        scalar=avS[:, q, C:C + 1], in1=avS[:, q, 0:C],
                        op0=ALU.mult, op1=ALU.subtract)
                    nvs = fpool.tile([P, C], dt.float32, name="nvs", tag="nvs",
                                     bufs=2)
                    if q % 2 == 1:
                        nc.scalar.activation(
                            nvs[:, :], nv[:, :], AF.Square,
                            accum_out=ssv[:, q:q + 1])
                    else:
                        nc.vector.scalar_tensor_tensor(
                            out=nvs[:, :], in0=nv[:, :], scalar=1.0, in1=nv[:, :],
                            op0=ALU.mult, op1=ALU.mult, accum_out=ssv[:, q:q + 1])
                nc.vector.tensor_scalar_max(ssv[:, :], ssv[:, :], 1e-24)
                nc.scalar.activation(lnv[:, :], ssv[:, :], AF.Ln)
                nc.scalar.activation(rnv[:, :], lnv[:, :], AF.Exp, scale=-0.5)
                ptb = [fpsum.tile([P, 4 * P], dt.bfloat16, name=f"ptb{h}",
                                  bufs=1) for h in range(2)]
                for q in range(4):
                    vbf = fpool.tile([P, C], dt.bfloat16, name="vbf",
                                     tag="vbf", bufs=2)
                    nc.vector.tensor_scalar(
                        out=vbf[:, :], in0=nvq[q][:, :],
                        scalar1=rnv[:, q:q + 1], scalar2=None, op0=ALU.mult)
                    for h in range(2):
                        nc.tensor.transpose(
                            ptb[h][:, q * P:(q + 1) * P],
                            vbf[:, h * P:(h + 1) * P],
                            ident_sb[:, :])
                # contiguous evacs; bank col layout is already 64*b + k, so
                # the FC reads lhsT with a single stride-64 AP — no permute
                for h in range(2):
                    nc.vector.tensor_copy(vT_all[:, h, :, :], ptb[h][:, :])

                # FC: out[8b, 128o] in 4 concurrent col-groups, separate banks
                # chunk j=(h,k): lhsT = vT_all[:, :, h, (j k)] -> [128, 4, 2]
                fcp = fcps.tile([P, OSL], dt.float32, name="fcp", bufs=1)
                NCH = K * C // P  # 128
                for j in range(NCH):
                    grp = j % 4
                    h, k = j % 2, j // 2
                    nc.tensor.matmul(
                        fcp[32 * grp:32 * grp + B, :],
                        lhsT=vT_all[:, h, :, k],
                        rhs=fwt_sb[:, j * OSL:(j + 1) * OSL],
                        start=(j < 4), stop=(j >= NCH - 4),
                        tile_position=(0, 32 * grp),
                        skip_group_check=True,
                    )
                sb4 = fpool.tile([P, OSL], dt.float32, name="sb4")
                nc.vector.memset(sb4[:, :], 0.0)
                for gq in range(4):
                    nc.scalar.copy(
                        sb4[32 * gq:32 * gq + B, :],
                        fcp[32 * gq:32 * gq + B, :])
                fcsum = fcps.tile([P, OSL], dt.float32, name="fcsum", bufs=1)
                nc.tensor.matmul(
                    fcsum[0:B, :], lhsT=sel_sb[:, :], rhs=sb4[:, :],
                    start=True, stop=True, skip_group_check=True,
                )
                fo = fpool.tile([B, OSL], dt.float32, name="fo")
                nc.vector.tensor_tensor(
                    out=fo[:, :], in0=fcsum[0:B, :], in1=fbb_sb[:, :],
                    op=ALU.add)

                # AllGather the [8, 128] slices + per-core partial sumsq
                fop = fpool.tile([B, OSL + 1], dt.float32, name="fop")
                nc.vector.scalar_tensor_tensor(
                    out=fop[:, 0:OSL], in0=fo[:, :], scalar=1.0,
                    in1=fo[:, :], op0=ALU.mult, op1=ALU.mult,
                    accum_out=fop[:, OSL:OSL + 1])
                nc.vector.tensor_copy(fop[:, 0:OSL], fo[:, :])
                ag_in = dram.tile([B, OSL + 1], dt.float32, name="ag_in")
                ag_out = dram.tile([NCORES * B, OSL + 1], dt.float32, name="ag_out")
                nc.sync.dma_start(out=ag_in[:, :], in_=fop[:, :])
                nc.gpsimd.collective_compute(
                    "AllGather",
                    ALU.bypass,
                    replica_groups=[list(range(NCORES))],
                    ins=[ag_in[:, :]],
                    outs=[ag_out[:, :]],
                )
                fin = fpool.tile([B, OUT], dt.float32, name="fin")
                agv = ag_out.rearrange("(c b) o -> b c o", b=B)
                nc.sync.dma_start(
                    out=fin.rearrange("b (c o) -> b c o", c=NCORES),
                    in_=agv[:, :, 0:OSL],
                )
                ssfp = fpool.tile([B, NCORES], dt.float32, name="ssfp")
                nc.sync.dma_start(out=ssfp[:, :], in_=agv[:, :, OSL])
                ssf = fpool.tile([B, 1], dt.float32, name="ssf")
                lnf = fpool.tile([B, 1], dt.float32, name="lnf")
                rnf = fpool.tile([B, 1], dt.float32, name="rnf")
                nc.vector.tensor_reduce(
                    out=ssf[:, :], in_=ssfp[:, :],
                    axis=mybir.AxisListType.X, op=ALU.add)
                nc.vector.tensor_scalar_max(ssf[:, :], ssf[:, :], 1e-24)
                nc.scalar.activation(lnf[:, :], ssf[:, :], AF.Ln)
                nc.scalar.activation(rnf[:, :], lnf[:, :], AF.Exp, scale=-0.5)
                fout = fpool.tile([B, OUT], dt.float32, name="fout")
                nc.vector.tensor_scalar(
                    out=fout[:, :], in0=fin[:, :],
                    scalar1=rnf[:, 0:1], scalar2=None, op0=ALU.mult)
                nc.sync.dma_start(out=out_d[:, :], in_=fout[:, :])

    # Force every activation onto the one table set holding Exp+Ln+Square
    import types
    import bass_rust as _bass_rust
    from concourse.hw_specs import get_activation_tables
    import concourse.mybir as mybir2

    def _act_tables_one_set(self):
        has_activation = any(
            isinstance(i, mybir2.InstActivation)
            for b in self.main_func.blocks
            for i in b.instructions
        )
        if not has_activation:
            return
        tables = get_activation_tables(self.m.arch)
        pref = "natural_log_exp_and_others"
        mod = [(k, (v if k == pref else set())) for k, v in tables.items()]
        _bass_rust.insert_act_table_loads(self, mod)

    nc.insert_act_table_loads = types.MethodType(_act_tables_one_set, nc)

    nc.compile()
    return nc


# ----------------------------------------------------------------------------
# Host-side input assembly per core
# ----------------------------------------------------------------------------

def _make_in_maps(feat, batch_ids, conv_w, conv_b, centroids, fc_w, fc_b):
    core_feat, T, n_pad = _plan(feat, batch_ids)

    wt = np.ascontiguousarray(conv_w.T).astype(BF16)                # [256, 64]
    erep = np.exp(conv_b.astype(np.float32) - np.float32(conv_b.max())
                  + np.float32(SHIFT))
    erep_rep = np.broadcast_to(
        np.tile(erep.astype(BF16), G)[None, :], (P, G * K)).copy()  # [128, G*K]
    cent = np.concatenate([centroids, centroids], 0).astype(BF16)   # [128, 256]
    corr_all = np.zeros((B * K, 2), np.float32)
    for b in range(B):
        corr_all[b * K:(b + 1) * K] = _pad_correction(conv_w, conv_b, n_pad[b])
    ident = np.eye(P, dtype=np.float32).astype(BF16)
    sel = np.zeros((P, B), np.float32)
    for gq in range(4):
        for b in range(B):
            sel[32 * gq + b, b] = 1.0

    OSL = OUT // NCORES
    in_maps = []
    for i in range(NCORES):
        cf = core_feat[i]
        featN = np.empty((P, T, C + 1), dtype=FP8)
        featN[:, :, 0:C] = cf.reshape(T, P, C).transpose(1, 0, 2).astype(FP8)
        featN[:, :, C] = FP8(1.0)
        featT = np.ascontiguousarray(cf.T).astype(BF16)
        # fc slice, negated, chunk-swizzled: chunk j=(h,k) covers
        # kc = k*256 + h*128 + p  -> fwt[p, j*128+o] = -fc_w[o_base+o, kc]
        fsl = -fc_w[i * OSL:(i + 1) * OSL]                          # [128, 16384]
        f4 = fsl.reshape(OSL, K, 2, P)                              # [o, k, h, p]
        fsw = np.ascontiguousarray(
            f4.transpose(3, 2, 1, 0).reshape(P, 2, K, OSL)          # [p, h, k, o]
             .transpose(0, 2, 1, 3)                                 # [p, k, h, o]
        )
        # chunk order j: j%2 = h, j//2 = k -> layout [p, (k h) o]
        fsw = fsw.reshape(P, K * C).astype(BF16)
        fbb = np.broadcast_to(fc_b[i * OSL:(i + 1) * OSL].astype(np.float32),
                              (B, OSL)).copy()
        in_maps.append({
            "featN": featN,
            "featT": featT,
            "wt": wt,
            "erep": erep_rep,
            "cent": cent,
            "corr": corr_all,
            "fwt": fsw,
            "fbb": fbb,
            "ident": ident,
            "sel": sel,
        })
    return in_maps, T


def _ensure_profile_hook():
    import sys
    import types
    try:
        from antenv.axon_hooks import get_axon_ntff_profile_hook  # noqa: F401
        return True
    except ImportError:
        pass
    try:
        from trn_agent_boot.trn_boot import _ntff_profile_via_ctypes
        hook = _ntff_profile_via_ctypes("/opt/axon/libaxon_pjrt.so")
        if hook is None:
            return False
        mod = types.ModuleType("antenv.axon_hooks")
        mod._hook = hook
        mod.get_axon_ntff_profile_hook = lambda: mod._hook
        mod.set_axon_ntff_profile_hook = lambda h: setattr(mod, "_hook", h)
        import antenv
        antenv.axon_hooks = mod
        sys.modules["antenv.axon_hooks"] = mod
        return True
    except Exception:
        return False


def kernel(feat, batch_ids, centroids, conv_w, conv_b, fc_w, fc_b, batch_size):
    from concourse.bass_utils import run_bass_kernel_spmd

    feat = np.asarray(feat, dtype=np.float32)
    batch_ids = np.asarray(batch_ids, dtype=np.int32)
    centroids = np.asarray(centroids, dtype=np.float32)
    conv_w = np.asarray(conv_w, dtype=np.float32)
    conv_b = np.asarray(conv_b, dtype=np.float32)
    fc_w = np.asarray(fc_w, dtype=np.float32)
    fc_b = np.asarray(fc_b, dtype=np.float32)

    assert conv_b.max() - conv_b.min() < 125.0, "conv_b spread too wide for SHIFT"

    in_maps, T = _make_in_maps(
        feat, batch_ids, conv_w, conv_b, centroids, fc_w, fc_b)

    if T not in _compiled_cache:
        _compiled_cache[T] = _build_nc(T)
    nc = _compiled_cache[T]

    global LAST_RESULT
    do_trace = PROFILE and _ensure_profile_hook()
    import os as _os
    _tc = _os.environ.get("TRACE_CORE")
    _kw = {"trace_cores": [int(_tc)]} if _tc else {}
    res = run_bass_kernel_spmd(
        nc, in_maps, core_ids=list(range(NCORES)), trace=do_trace, **_kw)
    LAST_RESULT = res
    return np.asarray(res.results[0]["out"], dtype=np.float32)
